# revision 41
# baseline (speedup 1.0000x reference)
"""Trainium2 Bass kernel for column-softmax attention.

reference semantics:
    scores = einsum('bqd,bkd->bqk', q, k) / sqrt(128)   # [B, Nq, Nk]
    attn   = softmax(scores, axis=1)                     # over the QUERY axis
    out    = einsum('bqk,bkd->bqd', attn, v)             # [B, Nq, D]

Because the softmax is over q, each key column k normalizes independently:
    out[q, d] = sum_k E[k, q] * r[k] * v[k, d],  E = exp(scores.T), r = 1/sum_q E[k, q]

Sharding: 8 cores = 4 batches x 2 key-halves.  Each core computes the partial
sum over its 2048 keys; the host adds the two partials per batch.

On-chip layout: the host pre-transposes Q and K to [D, N] (contraction dim on
partitions) and the kernel emits out.T [D, Nq]; the host transposes back.  The
softmax denominator is folded into V row-scaling so the normalize step touches
only 128x128 tiles per key tile.

The ScalarE exp pass (8.4M elements/core) is the roofline: 64 ACTIVATE
instructions of N=1024 from PSUM = ~66us engine-busy, the largest chunk the
8-bank PSUM allows while keeping a double-buffered score tile (4 banks) AND a
query-half output accumulator (4 banks) resident.  Structure:
  - row sums moved OFF ScalarE (accum_out costs ~190-280ns per ACTIVATE):
    per key tile a 3-stage fp16 pairwise-add chain + short reduce on DVE
    (~2.9us, fits under the 4.15us ACT tile period).  Last tile keeps
    accum_out so vsc15 is ready immediately for the tail.
  - startup: first q chunk + first key tile DMA'd first; a dummy exp preloads
    the ACT table and a 5-matmul memset warm-up stream releases the PE HAM
    clock-gate, all under the initial DMA window.
  - phase A (per key tile): scores matmul -> exp -> E resident fp16; previous
    tile's out.T contribution for query half A accumulated in PSUM (hides the
    second GEMM under the ScalarE exp span).
  - tail: query half B accumulated in the freed S-pool PSUM banks as FOUR
    sequential 512-col bank groups, each flushed + DMA'd under the next
    group's matmul stream (alternating the two PSUM tiles between groups --
    a same-tile write-after-read of the flush copy stalls ~0.9us); the first
    group's tiles 0..14 fill the window while the vsc15 chain resolves, with
    the half-A close slotted in before its kt=15; fp16 staging halves the
    out DMA; only the last group's short copy->DMA chain remains at the end.
PE weight-load runs are kept contiguous per stationary via explicit
ordering-only deps (the scheduler would otherwise split runs).
Measured: TimelineSim 110.8us baseline -> 90.4us.  Axon loop-difference
(back-to-back A/B, R=257 sustained): baseline 110.1us -> ~92us; quiet-window
R=129 burst readings ~79-81us vs the 95.3us baseline quote (~15-17% either way).
"""

import numpy as np

import concourse.bass as bass
import concourse.mybir as mybir
import concourse.tile as tile
from concourse.bass_utils import run_bass_kernel_spmd
from concourse.tile_rust import add_dep_helper

B, N, D = 4, 4096, 128
P = 128
NK = 2048                 # keys per core (half of 4096)
KT_TILES = NK // P        # 16 key tiles of 128
SCALE = 1.0 / np.sqrt(128.0)

F32 = mybir.dt.float32
F16 = mybir.dt.float16


def emit_body(nc, tc, pools, aps):
    big, epool, small, spsum, opsum = pools
    qt_d, kt_d, v_d, out_d = aps

    qT = big.tile([P, N], F16, tag="qT")            # [d, q]
    kT = big.tile([P, NK], F16, tag="kT")           # [d, k]
    vsb = big.tile([P, KT_TILES, D], F16, tag="v")  # [k_in_tile, k_tile, d]
    oacc = big.tile([P, N], F16, tag="oacc")        # [d, q] fp16 staging

    # DMA order: the first scores matmul needs kT tile 0 + qT[0:512] only.
    nc.sync.dma_start(qT[:, 0:512], qt_d[:, 0:512])
    nc.sync.dma_start(kT[:, 0:P], kt_d[:, 0:P])
    nc.sync.dma_start(qT[:, 512:1024], qt_d[:, 512:1024])
    for _qc in range(1, 4):
        nc.sync.dma_start(
            qT[:, _qc * 1024 : (_qc + 1) * 1024], qt_d[:, _qc * 1024 : (_qc + 1) * 1024]
        )
    # kT tiles 1.. are first needed at tile-1 scores (~7us in), q chunks first.
    nc.sync.dma_start(kT[:, P:NK], kt_d[:, P:NK])
    nc.sync.dma_start(vsb[:], v_d.rearrange("(t p) d -> p t d", p=P))

    # ACT table preload: dummy exp on a memset tile runs under the DMA window,
    # so the ~1.3us exp_and_others table load is off the critical path.
    warm_in = small.tile([P, 1], F32, tag="warm_in")
    warm_out = small.tile([P, 1], F32, tag="warm_out")
    nc.vector.memset(warm_in[:], 0.0)
    nc.scalar.activation(
        out=warm_out[:], in_=warm_in[:], func=mybir.ActivationFunctionType.Exp
    )

    # PE warm-up: a stream of matmuls on memset data runs during the
    # otherwise-idle DMA window, so the HAM clock-gate (cold 1.2GHz ->
    # warm 2.4GHz after ~3.4us of sustained PE activity) is released
    # before the first real scores matmul arrives.
    wsrc = big.tile([P, 512], F16, tag="wsrc")
    nc.vector.memset(wsrc[:], 0.0)
    Swarm = spsum.tile([P, 1024], F32, tag="S")
    # 3 warm matmuls: enough to keep PE activity continuous into phase A
    # (HAM un-throttles on sustained busy), few enough that the stream ends
    # before the first q/k data lands (PE is in-order, so extra warm matmuls
    # would DELAY the first real scores matmul).
    for _w in range(3):
        nc.tensor.matmul(
            Swarm[:, 0:512], lhsT=wsrc[:, 0:P], rhs=wsrc[:], start=True, stop=True
        )

    e_tiles = []
    v_tiles = []
    # Output accumulators for query half A (cols 0..2047) are built up during
    # phase A so the second GEMM's first half hides under the exp span.
    oa_tiles = []
    for _oc in range(2):
        O_a = opsum.tile([P, 1024], F32, tag="O")
        oa_tiles.append(O_a)

    pending_g2a = None
    for kt in range(KT_TILES):
        last_tile = kt == KT_TILES - 1
        E = epool.tile([P, N], F16, tag=f"E{kt}")   # [k, q] = exp(scores.T)
        rs = small.tile([P, 4], F32, tag="rs")
        last_g1 = None
        for h in range(4):
            S = spsum.tile([P, 1024], F32, tag="S")
            for u in range(2):
                last_g1 = nc.tensor.matmul(
                    S[:, u * 512 : (u + 1) * 512],
                    lhsT=kT[:, kt * P : (kt + 1) * P],
                    rhs=qT[:, h * 1024 + u * 512 : h * 1024 + u * 512 + 512],
                    start=True,
                    stop=True,
                )
            # For the LAST tile only, fuse the row sums into the activation
            # (accum_out, ~190-280ns extra ScalarE per chunk): the DVE reduce
            # chain would otherwise put vsc15 ~3us after the last exp, past
            # the point where the tail's half-A close needs it.
            nc.scalar.activation(
                out=E[:, h * 1024 : (h + 1) * 1024],
                in_=S[:],
                func=mybir.ActivationFunctionType.Exp,
                scale=float(SCALE),
                accum_out=rs[:, h : h + 1] if last_tile else None,
            )
        rsum = small.tile([P, 1], F32, tag="rsum")
        recip = small.tile([P, 1], F32, tag="recip")
        vsc = small.tile([P, D], F16, tag=f"vsc{kt}")  # [k, d] * r[k]
        # Row sums on DVE, off the ScalarE path (accum_out would cost an
        # extra ~190-280ns of ScalarE per ACTIVATE).  A straight TensorReduce
        # over [128, 4096] runs at 1 elem/cycle = ~4.3us/tile and would make
        # DVE the per-tile bottleneck; fp16 pairwise adds get the 2x DVE
        # rate.  (tensor_tensor_reduce would be one instruction but does not
        # compile on this toolchain: "ISA wrong length".)
        if last_tile:
            nc.vector.reduce_sum(out=rsum[:], in_=rs[:], axis=mybir.AxisListType.X)
        else:
            t1 = small.tile([P, 2048], F16, tag="rt1")
            t2 = small.tile([P, 1024], F16, tag="rt2")
            t3 = small.tile([P, 512], F16, tag="rt3")
            nc.vector.tensor_add(t1[:], E[:, 0:2048], E[:, 2048:4096])
            nc.vector.tensor_add(t2[:], t1[:, 0:1024], t1[:, 1024:2048])
            nc.vector.tensor_add(t3[:], t2[:, 0:512], t2[:, 512:1024])
            nc.vector.reduce_sum(out=rsum[:], in_=t3[:], axis=mybir.AxisListType.X)
        nc.vector.reciprocal(recip[:], rsum[:])
        nc.vector.tensor_scalar_mul(vsc[:], vsb[:, kt, :], recip[:])
        e_tiles.append(E)
        v_tiles.append(vsc)

        # Emit the PREVIOUS key tile's half-A output matmuls here, ordered
        # after this tile's scores matmuls (ordering-only deps).  This keeps
        # each kT weight-load run contiguous.
        if kt > 0:
            pv, pe_t, pkt = pending_g2a
            for oc in range(2):
                for u in range(2):
                    last_oa_mm = nc.tensor.matmul(
                        oa_tiles[oc][:, u * 512 : (u + 1) * 512],
                        lhsT=pv[:],
                        rhs=pe_t[:, oc * 1024 + u * 512 : oc * 1024 + (u + 1) * 512],
                        start=(pkt == 0),
                        stop=False,
                    )
                    if last_g1 is not None:
                        add_dep_helper(
                            last_oa_mm.ins,
                            last_g1.ins,
                            sync=False,
                            reason="keep kT weight-load run contiguous",
                        )
        pending_g2a = (vsc, E, kt)

    # Flush one [128, 1024] accumulator: two 512-col copies split across ACT
    # (idle in the tail) and DVE, each quarter DMA'd as soon as it is staged.
    # fp16 staging halves the out DMA bytes (the host adds the two per-batch
    # partials in fp32).
    def flush_oc(o_tile, lo):
        for qr in range(2):
            src = o_tile[:, qr * 512 : (qr + 1) * 512]
            dst = oacc[:, lo + qr * 512 : lo + (qr + 1) * 512]
            if qr == 0:
                nc.scalar.copy(out=dst, in_=src)
            else:
                nc.vector.tensor_copy(out=dst, in_=src)
            nc.sync.dma_start(out_d[:, lo + qr * 512 : lo + (qr + 1) * 512], dst)

    # Tail: query half B accumulates in the S-pool banks (free once the last
    # exp has read them) so it does NOT wait on the half-A flush.  The four
    # 512-col bank groups run SEQUENTIALLY (16 matmuls over all key tiles
    # each) so every group's flush copy + DMA hides under the next group's
    # matmul stream and only the last group's short flush chain remains at
    # the very end.  Costs extra vsc weight-load runs (hidden under the
    # matmul stream).
    ob_tiles = []
    for _oc in range(2):
        O_b = spsum.tile([P, 1024], F32, tag="S")
        ob_tiles.append(O_b)
    prev_mm = last_oa_mm
    # Group order alternates the two PSUM tiles so each quarter's flush copy
    # (a read of the tile) finishes under a DIFFERENT tile's matmul stream --
    # same-tile write-after-read would stall the next group ~0.9us.
    for qt4, (oc, u) in enumerate([(0, 0), (1, 0), (0, 1), (1, 1)]):
        for kt in range(KT_TILES):
            if qt4 == 0 and kt == KT_TILES - 1:
                # The first group's tiles 0..14 fill the ~1us window while the
                # vsc15 chain (accum -> reduce -> recip -> scale) resolves;
                # the half-A close slots in here, then the group's own kt=15.
                pv, pe_t, pkt = pending_g2a
                for coc in range(2):
                    for cu in range(2):
                        cmm = nc.tensor.matmul(
                            oa_tiles[coc][:, cu * 512 : (cu + 1) * 512],
                            lhsT=pv[:],
                            rhs=pe_t[:, coc * 1024 + cu * 512 : coc * 1024 + (cu + 1) * 512],
                            start=False,
                            stop=True,
                        )
                        add_dep_helper(
                            cmm.ins, prev_mm.ins, sync=False,
                            reason="half-A close inside first tail group",
                        )
                        prev_mm = cmm
            mm = nc.tensor.matmul(
                ob_tiles[oc][:, u * 512 : (u + 1) * 512],
                lhsT=v_tiles[kt][:],
                rhs=e_tiles[kt][:, 2048 + oc * 1024 + u * 512 : 2048 + oc * 1024 + (u + 1) * 512],
                start=(kt == 0),
                stop=(kt == KT_TILES - 1),
            )
            add_dep_helper(
                mm.ins, prev_mm.ins, sync=False,
                reason="sequential bank groups in tail",
            )
            prev_mm = mm
        # This quarter's flush + DMA hide under the next quarter's matmuls.
        lo = 2048 + oc * 1024 + u * 512
        src = ob_tiles[oc][:, u * 512 : (u + 1) * 512]
        dst = oacc[:, lo : lo + 512]
        if qt4 % 2 == 0:
            nc.scalar.copy(out=dst, in_=src)
        else:
            nc.vector.tensor_copy(out=dst, in_=src)
        nc.sync.dma_start(out_d[:, lo : lo + 512], dst)
        if qt4 == 0:
            flush_oc(oa_tiles[0], 0)
            flush_oc(oa_tiles[1], 1024)


def build_bass(repeat=1, loop=False):
    nc = bass.Bass("TRN2", target_bir_lowering=False, debug=False)
    qt_d = nc.dram_tensor("qt", [P, N], F16, kind="ExternalInput").ap()
    kt_d = nc.dram_tensor("kt", [P, NK], F16, kind="ExternalInput").ap()
    v_d = nc.dram_tensor("v", [NK, D], F16, kind="ExternalInput").ap()
    out_d = nc.dram_tensor("out_t", [P, N], F16, kind="ExternalOutput").ap()

    with tile.TileContext(nc) as tc:
        with (
            tc.tile_pool(name="big", bufs=1) as big,
            tc.tile_pool(name="epool", bufs=1) as epool,
            tc.tile_pool(name="small", bufs=2) as small,
            tc.tile_pool(name="spsum", bufs=2, space="PSUM") as spsum,
            tc.tile_pool(name="opsum", bufs=2, space="PSUM") as opsum,
        ):
            def body():
                emit_body(
                    nc,
                    tc,
                    (big, epool, small, spsum, opsum),
                    (qt_d, kt_d, v_d, out_d),
                )

            if loop and repeat > 1:
                with tc.For_i(
                    0, repeat, 1,
                    hint_engines=(mybir.EngineType.PE, mybir.EngineType.Activation),
                ):
                    body()
            else:
                for _ in range(repeat):
                    body()
    return nc


def legalize_waits(nc, max_waits=1):
    """Hoist excess semaphore waits into standalone EventSemaphore ops.

    The walrus codegen for several engine instruction structs accepts only a
    single sync-wait command; Tile sometimes emits more.  Executing the extra
    waits in a preceding same-engine EventSemaphore is semantically identical
    (the engine runs its stream in order).
    """
    for fn in nc.m.functions:
        for blk in fn.blocks:
            out = []
            for inst in blk.instructions:
                si = inst.sync_info
                if (
                    si is not None
                    and si.on_wait
                    and len(si.on_wait) > max_waits
                    and inst.opcode != "EventSemaphore"
                ):
                    waits = list(si.on_wait)
                    extra, keep = waits[:-max_waits], waits[-max_waits:]
                    for n, w in enumerate(extra):
                        out.append(
                            mybir.InstEventSemaphore(
                                name=f"{inst.name}_prewait{n}",
                                engine=inst.engine,
                                ins=[],
                                outs=[],
                                sync_info=mybir.SyncInfo(on_wait=[w], on_update=[]),
                            )
                        )
                    si.on_wait = keep
                out.append(inst)
            blk.instructions = out
    return nc


_NC_CACHE = {}


def _get_nc(repeat=1, **kw):
    key = ("nc", repeat, tuple(sorted(kw.items())))
    if key not in _NC_CACHE:
        _NC_CACHE[key] = legalize_waits(build_bass(repeat, **kw))
    return _NC_CACHE[key]


def kernel(q, k, v):
    q = np.asarray(q, dtype=np.float32)
    k = np.asarray(k, dtype=np.float32)
    v = np.asarray(v, dtype=np.float32)

    in_maps = []
    for c in range(8):
        b, h = c // 2, c % 2
        in_maps.append(
            {
                "qt": np.ascontiguousarray(q[b].T).astype(np.float16),
                "kt": np.ascontiguousarray(k[b, h * NK : (h + 1) * NK].T).astype(np.float16),
                "v": np.ascontiguousarray(v[b, h * NK : (h + 1) * NK]).astype(np.float16),
            }
        )

    nc = _get_nc()
    res = run_bass_kernel_spmd(nc, in_maps, list(range(8))).results

    out = np.empty((B, N, D), dtype=np.float32)
    for b in range(B):
        out[b] = (
            res[2 * b]["out_t"].astype(np.float32)
            + res[2 * b + 1]["out_t"].astype(np.float32)
        ).T
    return out


# revision 45
# speedup vs baseline: 1.0082x; 1.0082x over previous
"""Trainium2 Bass kernel for column-softmax attention.

reference semantics:
    scores = einsum('bqd,bkd->bqk', q, k) / sqrt(128)   # [B, Nq, Nk]
    attn   = softmax(scores, axis=1)                     # over the QUERY axis
    out    = einsum('bqk,bkd->bqd', attn, v)             # [B, Nq, D]

Because the softmax is over q, each key column k normalizes independently:
    out[q, d] = sum_k E[k, q] * r[k] * v[k, d],  E = exp(scores.T), r = 1/sum_q E[k, q]

Sharding: 8 cores = 4 batches x 2 key-halves.  Each core computes the partial
sum over its 2048 keys; the host adds the two partials per batch.

On-chip layout: the host pre-transposes Q and K to [D, N] (contraction dim on
partitions) and the kernel emits out.T [D, Nq]; the host transposes back.  The
softmax denominator is folded into V row-scaling so the normalize step touches
only 128x128 tiles per key tile.

The ScalarE exp pass (8.4M elements/core) is the roofline: 64 ACTIVATE
instructions of N=1024 from PSUM = ~66us engine-busy, the largest chunk the
8-bank PSUM allows while keeping a double-buffered score tile (4 banks) AND a
query-half output accumulator (4 banks) resident.  Structure:
  - row sums moved OFF ScalarE (accum_out costs ~190-280ns per ACTIVATE):
    per key tile a 3-stage fp16 pairwise-add chain + short reduce on DVE
    (~2.9us, fits under the 4.15us ACT tile period).  Last tile keeps
    accum_out so vsc15 is ready immediately for the tail.
  - startup: first q chunk + first key tile DMA'd first; a dummy exp preloads
    the ACT table and a 5-matmul memset warm-up stream releases the PE HAM
    clock-gate, all under the initial DMA window.
  - phase A (per key tile): scores matmul -> exp -> E resident fp16; previous
    tile's out.T contribution for query half A accumulated in PSUM (hides the
    second GEMM under the ScalarE exp span).
  - tail: query half B accumulated in the freed S-pool PSUM banks as FOUR
    sequential 512-col bank groups, each flushed + DMA'd under the next
    group's matmul stream (alternating the two PSUM tiles between groups --
    a same-tile write-after-read of the flush copy stalls ~0.9us); the first
    group's tiles 0..14 fill the window while the vsc15 chain resolves, with
    the half-A close slotted in before its kt=15; fp16 staging halves the
    out DMA; only the last group's short copy->DMA chain remains at the end.
PE weight-load runs are kept contiguous per stationary via explicit
ordering-only deps (the scheduler would otherwise split runs).
Measured: TimelineSim 110.8us baseline -> 90.4us.  Axon loop-difference
(back-to-back A/B, R=257 sustained): baseline 110.1us -> ~92us; quiet-window
R=129 burst readings ~79-81us vs the 95.3us baseline quote (~15-17% either way).
"""

import numpy as np

import concourse.bass as bass
import concourse.mybir as mybir
import concourse.tile as tile
from concourse.bass_utils import run_bass_kernel_spmd
from concourse.tile_rust import add_dep_helper

B, N, D = 4, 4096, 128
P = 128
NK = 2048                 # keys per core (half of 4096)
KT_TILES = NK // P        # 16 key tiles of 128
SCALE = 1.0 / np.sqrt(128.0)

F32 = mybir.dt.float32
F16 = mybir.dt.float16


def emit_body(nc, tc, pools, aps):
    big, epool, small, spsum, opsum = pools
    qt_d, kt_d, v_d, out_d = aps

    qT = big.tile([P, N], F16, tag="qT")            # [d, q]
    kT = big.tile([P, NK], F16, tag="kT")           # [d, k]
    vsb = big.tile([P, KT_TILES, D], F16, tag="v")  # [k_in_tile, k_tile, d]
    oacc = big.tile([P, N], F16, tag="oacc")        # [d, q] fp16 staging

    # DMA order: the first scores matmul needs kT tile 0 + qT[0:512] only.
    nc.sync.dma_start(qT[:, 0:512], qt_d[:, 0:512])
    nc.sync.dma_start(kT[:, 0:P], kt_d[:, 0:P])
    nc.sync.dma_start(qT[:, 512:1024], qt_d[:, 512:1024])
    for _qc in range(1, 4):
        nc.sync.dma_start(
            qT[:, _qc * 1024 : (_qc + 1) * 1024], qt_d[:, _qc * 1024 : (_qc + 1) * 1024]
        )
    # kT tiles 1.. are first needed at tile-1 scores (~7us in), q chunks first.
    nc.sync.dma_start(kT[:, P:NK], kt_d[:, P:NK])
    nc.sync.dma_start(vsb[:], v_d.rearrange("(t p) d -> p t d", p=P))

    # ACT table preload: dummy exp on a memset tile runs under the DMA window,
    # so the ~1.3us exp_and_others table load is off the critical path.
    warm_in = small.tile([P, 1], F32, tag="warm_in")
    warm_out = small.tile([P, 1], F32, tag="warm_out")
    nc.vector.memset(warm_in[:], 0.0)
    nc.scalar.activation(
        out=warm_out[:], in_=warm_in[:], func=mybir.ActivationFunctionType.Exp
    )

    # PE warm-up: a stream of matmuls on memset data runs during the
    # otherwise-idle DMA window, so the HAM clock-gate (cold 1.2GHz ->
    # warm 2.4GHz after ~3.4us of sustained PE activity) is released
    # before the first real scores matmul arrives.
    wsrc = big.tile([P, 512], F16, tag="wsrc")
    nc.vector.memset(wsrc[:], 0.0)
    Swarm = spsum.tile([P, 1024], F32, tag="S")
    # 3 warm matmuls: enough to keep PE activity continuous into phase A
    # (HAM un-throttles on sustained busy), few enough that the stream ends
    # before the first q/k data lands (PE is in-order, so extra warm matmuls
    # would DELAY the first real scores matmul).
    for _w in range(3):
        nc.tensor.matmul(
            Swarm[:, 0:512], lhsT=wsrc[:, 0:P], rhs=wsrc[:], start=True, stop=True
        )

    e_tiles = []
    v_tiles = []
    # Output accumulators for query half A (cols 0..2047) are built up during
    # phase A so the second GEMM's first half hides under the exp span.
    oa_tiles = []
    for _oc in range(2):
        O_a = opsum.tile([P, 1024], F32, tag="O")
        oa_tiles.append(O_a)

    pending_g2a = None
    for kt in range(KT_TILES):
        last_tile = kt == KT_TILES - 1
        E = epool.tile([P, N], F16, tag=f"E{kt}")   # [k, q] = exp(scores.T)
        rs = small.tile([P, 4], F32, tag="rs")
        last_g1 = None
        for h in range(4):
            S = spsum.tile([P, 1024], F32, tag="S")
            for u in range(2):
                last_g1 = nc.tensor.matmul(
                    S[:, u * 512 : (u + 1) * 512],
                    lhsT=kT[:, kt * P : (kt + 1) * P],
                    rhs=qT[:, h * 1024 + u * 512 : h * 1024 + u * 512 + 512],
                    start=True,
                    stop=True,
                )
            # For the LAST tile only, fuse the row sums into the activation
            # (accum_out, ~190-280ns extra ScalarE per chunk): the DVE reduce
            # chain would otherwise put vsc15 ~3us after the last exp, past
            # the point where the tail's half-A close needs it.
            nc.scalar.activation(
                out=E[:, h * 1024 : (h + 1) * 1024],
                in_=S[:],
                func=mybir.ActivationFunctionType.Exp,
                scale=float(SCALE),
                accum_out=rs[:, h : h + 1] if last_tile else None,
            )
        rsum = small.tile([P, 1], F32, tag="rsum")
        recip = small.tile([P, 1], F32, tag="recip")
        vsc = small.tile([P, D], F16, tag=f"vsc{kt}")  # [k, d] * r[k]
        # Row sums on DVE, off the ScalarE path (accum_out would cost an
        # extra ~190-280ns of ScalarE per ACTIVATE).  A straight TensorReduce
        # over [128, 4096] runs at 1 elem/cycle = ~4.3us/tile and would make
        # DVE the per-tile bottleneck; fp16 pairwise adds get the 2x DVE
        # rate.  (tensor_tensor_reduce would be one instruction but does not
        # compile on this toolchain: "ISA wrong length".)
        if last_tile:
            nc.vector.reduce_sum(out=rsum[:], in_=rs[:], axis=mybir.AxisListType.X)
        else:
            t1 = small.tile([P, 2048], F16, tag="rt1")
            t2 = small.tile([P, 1024], F16, tag="rt2")
            t3 = small.tile([P, 512], F16, tag="rt3")
            nc.vector.tensor_add(t1[:], E[:, 0:2048], E[:, 2048:4096])
            nc.vector.tensor_add(t2[:], t1[:, 0:1024], t1[:, 1024:2048])
            nc.vector.tensor_add(t3[:], t2[:, 0:512], t2[:, 512:1024])
            nc.vector.reduce_sum(out=rsum[:], in_=t3[:], axis=mybir.AxisListType.X)
        nc.vector.reciprocal(recip[:], rsum[:])
        nc.vector.tensor_scalar_mul(vsc[:], vsb[:, kt, :], recip[:])
        e_tiles.append(E)
        v_tiles.append(vsc)

        # Emit the PREVIOUS key tile's half-A output matmuls here, ordered
        # after this tile's scores matmuls (ordering-only deps).  This keeps
        # each kT weight-load run contiguous.
        if kt > 0:
            pv, pe_t, pkt = pending_g2a
            for oc in range(2):
                for u in range(2):
                    last_oa_mm = nc.tensor.matmul(
                        oa_tiles[oc][:, u * 512 : (u + 1) * 512],
                        lhsT=pv[:],
                        rhs=pe_t[:, oc * 1024 + u * 512 : oc * 1024 + (u + 1) * 512],
                        start=(pkt == 0),
                        stop=False,
                    )
                    if last_g1 is not None:
                        add_dep_helper(
                            last_oa_mm.ins,
                            last_g1.ins,
                            sync=False,
                            reason="keep kT weight-load run contiguous",
                        )
        pending_g2a = (vsc, E, kt)

    # Flush one [128, 1024] accumulator: two 512-col copies split across ACT
    # (idle in the tail) and DVE, each quarter DMA'd as soon as it is staged.
    # fp16 staging halves the out DMA bytes (the host adds the two per-batch
    # partials in fp32).
    def flush_oc(o_tile, lo):
        for qr in range(2):
            src = o_tile[:, qr * 512 : (qr + 1) * 512]
            dst = oacc[:, lo + qr * 512 : lo + (qr + 1) * 512]
            if qr == 0:
                nc.scalar.copy(out=dst, in_=src)
            else:
                nc.vector.tensor_copy(out=dst, in_=src)
            nc.sync.dma_start(out_d[:, lo + qr * 512 : lo + (qr + 1) * 512], dst)

    # Tail: query half B accumulates in the S-pool banks (free once the last
    # exp has read them) so it does NOT wait on the half-A flush.  The four
    # 512-col bank groups run SEQUENTIALLY (16 matmuls over all key tiles
    # each) so every group's flush copy + DMA hides under the next group's
    # matmul stream and only the last group's short flush chain remains at
    # the very end.  Costs extra vsc weight-load runs (hidden under the
    # matmul stream).
    ob_tiles = []
    for _oc in range(2):
        O_b = spsum.tile([P, 1024], F32, tag="S")
        ob_tiles.append(O_b)
    prev_mm = last_oa_mm
    # Group order alternates the two PSUM tiles so each quarter's flush copy
    # (a read of the tile) finishes under a DIFFERENT tile's matmul stream --
    # same-tile write-after-read would stall the next group ~0.9us.
    for qt4, (oc, u) in enumerate([(0, 0), (1, 0), (0, 1), (1, 1)]):
        for kt in range(KT_TILES):
            if qt4 == 0 and kt == KT_TILES - 1:
                # The first group's tiles 0..14 fill the ~1us window while the
                # vsc15 chain (accum -> reduce -> recip -> scale) resolves;
                # the half-A close slots in here, then the group's own kt=15.
                pv, pe_t, pkt = pending_g2a
                for coc in range(2):
                    for cu in range(2):
                        cmm = nc.tensor.matmul(
                            oa_tiles[coc][:, cu * 512 : (cu + 1) * 512],
                            lhsT=pv[:],
                            rhs=pe_t[:, coc * 1024 + cu * 512 : coc * 1024 + (cu + 1) * 512],
                            start=False,
                            stop=True,
                        )
                        add_dep_helper(
                            cmm.ins, prev_mm.ins, sync=False,
                            reason="half-A close inside first tail group",
                        )
                        prev_mm = cmm
            mm = nc.tensor.matmul(
                ob_tiles[oc][:, u * 512 : (u + 1) * 512],
                lhsT=v_tiles[kt][:],
                rhs=e_tiles[kt][:, 2048 + oc * 1024 + u * 512 : 2048 + oc * 1024 + (u + 1) * 512],
                start=(kt == 0),
                stop=(kt == KT_TILES - 1),
            )
            add_dep_helper(
                mm.ins, prev_mm.ins, sync=False,
                reason="sequential bank groups in tail",
            )
            prev_mm = mm
        # This quarter's flush + DMA hide under the next quarter's matmuls.
        lo = 2048 + oc * 1024 + u * 512
        src = ob_tiles[oc][:, u * 512 : (u + 1) * 512]
        dst = oacc[:, lo : lo + 512]
        if qt4 % 2 == 0:
            nc.scalar.copy(out=dst, in_=src)
        else:
            nc.vector.tensor_copy(out=dst, in_=src)
        nc.sync.dma_start(out_d[:, lo : lo + 512], dst)
        if qt4 == 0:
            flush_oc(oa_tiles[0], 0)
            flush_oc(oa_tiles[1], 1024)


def build_bass(repeat=1, loop=False):
    nc = bass.Bass("TRN2", target_bir_lowering=False, debug=False)
    qt_d = nc.dram_tensor("qt", [P, N], F16, kind="ExternalInput").ap()
    kt_d = nc.dram_tensor("kt", [P, NK], F16, kind="ExternalInput").ap()
    v_d = nc.dram_tensor("v", [NK, D], F16, kind="ExternalInput").ap()
    out_d = nc.dram_tensor("out_t", [P, N], F16, kind="ExternalOutput").ap()

    with tile.TileContext(nc) as tc:
        with (
            tc.tile_pool(name="big", bufs=1) as big,
            tc.tile_pool(name="epool", bufs=1) as epool,
            tc.tile_pool(name="small", bufs=2) as small,
            tc.tile_pool(name="spsum", bufs=2, space="PSUM") as spsum,
            tc.tile_pool(name="opsum", bufs=2, space="PSUM") as opsum,
        ):
            def body():
                emit_body(
                    nc,
                    tc,
                    (big, epool, small, spsum, opsum),
                    (qt_d, kt_d, v_d, out_d),
                )

            if loop and repeat > 1:
                with tc.For_i(
                    0, repeat, 1,
                    hint_engines=(mybir.EngineType.PE, mybir.EngineType.Activation),
                ):
                    body()
            else:
                for _ in range(repeat):
                    body()
    return nc


def legalize_waits(nc, max_waits=1):
    """Hoist excess semaphore waits into standalone EventSemaphore ops.

    The walrus codegen for several engine instruction structs accepts only a
    single sync-wait command; Tile sometimes emits more.  Executing the extra
    waits in a preceding same-engine EventSemaphore is semantically identical
    (the engine runs its stream in order).
    """
    for fn in nc.m.functions:
        for blk in fn.blocks:
            out = []
            for inst in blk.instructions:
                si = inst.sync_info
                if (
                    si is not None
                    and si.on_wait
                    and len(si.on_wait) > max_waits
                    and inst.opcode != "EventSemaphore"
                ):
                    waits = list(si.on_wait)
                    extra, keep = waits[:-max_waits], waits[-max_waits:]
                    for n, w in enumerate(extra):
                        out.append(
                            mybir.InstEventSemaphore(
                                name=f"{inst.name}_prewait{n}",
                                engine=inst.engine,
                                ins=[],
                                outs=[],
                                sync_info=mybir.SyncInfo(on_wait=[w], on_update=[]),
                            )
                        )
                    si.on_wait = keep
                out.append(inst)
            blk.instructions = out
    return nc


_NC_CACHE = {}


def _get_nc(repeat=1, **kw):
    key = ("nc", repeat, tuple(sorted(kw.items())))
    if key not in _NC_CACHE:
        _NC_CACHE[key] = legalize_waits(build_bass(repeat, **kw))
    return _NC_CACHE[key]


def kernel(q, k, v):
    q = np.asarray(q, dtype=np.float32)
    k = np.asarray(k, dtype=np.float32)
    v = np.asarray(v, dtype=np.float32)

    in_maps = []
    for c in range(8):
        b, h = c // 2, c % 2
        in_maps.append(
            {
                "qt": np.ascontiguousarray(q[b].T).astype(np.float16),
                "kt": np.ascontiguousarray(k[b, h * NK : (h + 1) * NK].T).astype(np.float16),
                "v": np.ascontiguousarray(v[b, h * NK : (h + 1) * NK]).astype(np.float16),
            }
        )

    nc = _get_nc()
    res = run_bass_kernel_spmd(nc, in_maps, list(range(8))).results

    out = np.empty((B, N, D), dtype=np.float32)
    for b in range(B):
        out[b] = (
            res[2 * b]["out_t"].astype(np.float32)
            + res[2 * b + 1]["out_t"].astype(np.float32)
        ).T
    return out


# revision 46
# speedup vs baseline: 1.0306x; 1.0222x over previous
"""Trainium2 Bass kernel for column-softmax attention.

reference semantics:
    scores = einsum('bqd,bkd->bqk', q, k) / sqrt(128)   # [B, Nq, Nk]
    attn   = softmax(scores, axis=1)                     # over the QUERY axis
    out    = einsum('bqk,bkd->bqd', attn, v)             # [B, Nq, D]

Because the softmax is over q, each key column k normalizes independently:
    out[q, d] = sum_k E[k, q] * r[k] * v[k, d],  E = exp(scores.T), r = 1/sum_q E[k, q]

Sharding: 8 cores = 4 batches x 2 key-halves.  Each core computes the partial
sum over its 2048 keys; the host adds the two partials per batch.

On-chip layout: the host pre-transposes Q and K to [D, N] (contraction dim on
partitions) and the kernel emits out.T [D, Nq]; the host transposes back.  The
softmax denominator is folded into V row-scaling so the normalize step touches
only 128x128 tiles per key tile.

The ScalarE exp pass (8.4M elements/core) is the roofline: 64 ACTIVATE
instructions of N=1024 from PSUM = ~66us engine-busy, the largest chunk the
8-bank PSUM allows while keeping a double-buffered score tile (4 banks) AND a
query-half output accumulator (4 banks) resident.  Structure:
  - row sums moved OFF ScalarE (accum_out costs ~190-280ns per ACTIVATE):
    per key tile a 3-stage fp16 pairwise-add chain + short reduce on DVE
    (~2.9us, fits under the 4.15us ACT tile period).  Last tile keeps
    accum_out so vsc15 is ready immediately for the tail.
  - startup: first q chunk + first key tile DMA'd first; a dummy exp preloads
    the ACT table and a 5-matmul memset warm-up stream releases the PE HAM
    clock-gate, all under the initial DMA window.
  - phase A (per key tile): scores matmul -> exp -> E resident fp16; previous
    tile's out.T contribution for query half A accumulated in PSUM (hides the
    second GEMM under the ScalarE exp span).
  - tail: query half B accumulated in the freed S-pool PSUM banks as FOUR
    sequential 512-col bank groups, each flushed + DMA'd under the next
    group's matmul stream (alternating the two PSUM tiles between groups --
    a same-tile write-after-read of the flush copy stalls ~0.9us); the first
    group's tiles 0..14 fill the window while the vsc15 chain resolves, with
    the half-A close slotted in before its kt=15; fp16 staging halves the
    out DMA; only the last group's short copy->DMA chain remains at the end.
PE weight-load runs are kept contiguous per stationary via explicit
ordering-only deps (the scheduler would otherwise split runs).
Measured: TimelineSim 110.8us baseline -> 90.4us.  Axon loop-difference
(back-to-back A/B, R=257 sustained): baseline 110.1us -> ~92us; quiet-window
R=129 burst readings ~79-81us vs the 95.3us baseline quote (~15-17% either way).
"""

import numpy as np

import concourse.bass as bass
import concourse.mybir as mybir
import concourse.tile as tile
from concourse.bass_utils import run_bass_kernel_spmd
from concourse.tile_rust import add_dep_helper

B, N, D = 4, 4096, 128
P = 128
NK = 2048                 # keys per core (half of 4096)
KT_TILES = NK // P        # 16 key tiles of 128
SCALE = 1.0 / np.sqrt(128.0)

F32 = mybir.dt.float32
F16 = mybir.dt.float16


def emit_body(nc, tc, pools, aps):
    big, epool, small, spsum, opsum = pools
    qt_d, kt_d, v_d, out_d = aps

    qT = big.tile([P, N], F16, tag="qT")            # [d, q]
    kT = big.tile([P, NK], F16, tag="kT")           # [d, k]
    vsb = big.tile([P, KT_TILES, D], F16, tag="v")  # [k_in_tile, k_tile, d]
    oacc = big.tile([P, N], F16, tag="oacc")        # [d, q] fp16 staging

    # DMA order: the first scores matmul needs kT tile 0 + qT[0:512] only.
    nc.sync.dma_start(qT[:, 0:512], qt_d[:, 0:512])
    nc.sync.dma_start(kT[:, 0:P], kt_d[:, 0:P])
    nc.sync.dma_start(qT[:, 512:1024], qt_d[:, 512:1024])
    for _qc in range(1, 4):
        nc.sync.dma_start(
            qT[:, _qc * 1024 : (_qc + 1) * 1024], qt_d[:, _qc * 1024 : (_qc + 1) * 1024]
        )
    # kT tiles 1.. are first needed at tile-1 scores (~7us in), q chunks first.
    nc.sync.dma_start(kT[:, P:NK], kt_d[:, P:NK])
    nc.sync.dma_start(vsb[:], v_d.rearrange("(t p) d -> p t d", p=P))

    # ACT table preload: dummy exp on a memset tile runs under the DMA window,
    # so the ~1.3us exp_and_others table load is off the critical path.
    warm_in = small.tile([P, 1], F32, tag="warm_in")
    warm_out = small.tile([P, 1], F32, tag="warm_out")
    nc.vector.memset(warm_in[:], 0.0)
    nc.scalar.activation(
        out=warm_out[:], in_=warm_in[:], func=mybir.ActivationFunctionType.Exp
    )

    # PE warm-up: a stream of matmuls on memset data runs during the
    # otherwise-idle DMA window, so the HAM clock-gate (cold 1.2GHz ->
    # warm 2.4GHz after ~3.4us of sustained PE activity) is released
    # before the first real scores matmul arrives.
    wsrc = big.tile([P, 512], F16, tag="wsrc")
    nc.vector.memset(wsrc[:], 0.0)
    Swarm = spsum.tile([P, 1024], F32, tag="S")
    # 3 warm matmuls: enough to keep PE activity continuous into phase A
    # (HAM un-throttles on sustained busy), few enough that the stream ends
    # before the first q/k data lands (PE is in-order, so extra warm matmuls
    # would DELAY the first real scores matmul).
    for _w in range(3):
        nc.tensor.matmul(
            Swarm[:, 0:512], lhsT=wsrc[:, 0:P], rhs=wsrc[:], start=True, stop=True
        )

    e_tiles = []
    v_tiles = []
    # Output accumulators for query half A (cols 0..2047) are built up during
    # phase A so the second GEMM's first half hides under the exp span.
    oa_tiles = []
    for _oc in range(2):
        O_a = opsum.tile([P, 1024], F32, tag="O")
        oa_tiles.append(O_a)

    pending_g2a = None
    for kt in range(KT_TILES):
        last_tile = kt == KT_TILES - 1
        E = epool.tile([P, N], F16, tag=f"E{kt}")   # [k, q] = exp(scores.T)
        rs = small.tile([P, 4], F32, tag="rs")
        last_g1 = None
        for h in range(4):
            S = spsum.tile([P, 1024], F32, tag="S")
            for u in range(2):
                last_g1 = nc.tensor.matmul(
                    S[:, u * 512 : (u + 1) * 512],
                    lhsT=kT[:, kt * P : (kt + 1) * P],
                    rhs=qT[:, h * 1024 + u * 512 : h * 1024 + u * 512 + 512],
                    start=True,
                    stop=True,
                )
            # For the LAST tile only, fuse the row sums into the activation
            # (accum_out, ~190-280ns extra ScalarE per chunk): the DVE reduce
            # chain would otherwise put vsc15 ~3us after the last exp, past
            # the point where the tail's half-A close needs it.
            nc.scalar.activation(
                out=E[:, h * 1024 : (h + 1) * 1024],
                in_=S[:],
                func=mybir.ActivationFunctionType.Exp,
                scale=float(SCALE),
                accum_out=rs[:, h : h + 1] if last_tile else None,
            )
        rsum = small.tile([P, 1], F32, tag="rsum")
        recip = small.tile([P, 1], F32, tag="recip")
        vsc = small.tile([P, D], F16, tag=f"vsc{kt}")  # [k, d] * r[k]
        # Row sums on DVE, off the ScalarE path (accum_out would cost an
        # extra ~190-280ns of ScalarE per ACTIVATE).  A straight TensorReduce
        # over [128, 4096] runs at 1 elem/cycle = ~4.3us/tile and would make
        # DVE the per-tile bottleneck; fp16 pairwise adds get the 2x DVE
        # rate.  (tensor_tensor_reduce would be one instruction but does not
        # compile on this toolchain: "ISA wrong length".)
        if last_tile:
            nc.vector.reduce_sum(out=rsum[:], in_=rs[:], axis=mybir.AxisListType.X)
        else:
            t1 = small.tile([P, 2048], F16, tag="rt1")
            t2 = small.tile([P, 1024], F16, tag="rt2")
            t3 = small.tile([P, 512], F16, tag="rt3")
            nc.vector.tensor_add(t1[:], E[:, 0:2048], E[:, 2048:4096])
            nc.vector.tensor_add(t2[:], t1[:, 0:1024], t1[:, 1024:2048])
            nc.vector.tensor_add(t3[:], t2[:, 0:512], t2[:, 512:1024])
            nc.vector.reduce_sum(out=rsum[:], in_=t3[:], axis=mybir.AxisListType.X)
        nc.vector.reciprocal(recip[:], rsum[:])
        nc.vector.tensor_scalar_mul(vsc[:], vsb[:, kt, :], recip[:])
        e_tiles.append(E)
        v_tiles.append(vsc)

        # Emit the PREVIOUS key tile's half-A output matmuls here, ordered
        # after this tile's scores matmuls (ordering-only deps).  This keeps
        # each kT weight-load run contiguous.
        if kt > 0:
            pv, pe_t, pkt = pending_g2a
            for oc in range(2):
                for u in range(2):
                    last_oa_mm = nc.tensor.matmul(
                        oa_tiles[oc][:, u * 512 : (u + 1) * 512],
                        lhsT=pv[:],
                        rhs=pe_t[:, oc * 1024 + u * 512 : oc * 1024 + (u + 1) * 512],
                        start=(pkt == 0),
                        stop=False,
                    )
                    if last_g1 is not None:
                        add_dep_helper(
                            last_oa_mm.ins,
                            last_g1.ins,
                            sync=False,
                            reason="keep kT weight-load run contiguous",
                        )
        pending_g2a = (vsc, E, kt)

    # Flush one [128, 1024] accumulator: two 512-col copies split across ACT
    # (idle in the tail) and DVE, each quarter DMA'd as soon as it is staged.
    # fp16 staging halves the out DMA bytes (the host adds the two per-batch
    # partials in fp32).
    def flush_oc(o_tile, lo):
        for qr in range(2):
            src = o_tile[:, qr * 512 : (qr + 1) * 512]
            dst = oacc[:, lo + qr * 512 : lo + (qr + 1) * 512]
            if qr == 0:
                nc.scalar.copy(out=dst, in_=src)
            else:
                nc.vector.tensor_copy(out=dst, in_=src)
            nc.sync.dma_start(out_d[:, lo + qr * 512 : lo + (qr + 1) * 512], dst)

    # Tail: query half B accumulates in the S-pool banks (free once the last
    # exp has read them) so it does NOT wait on the half-A flush.  The four
    # 512-col bank groups run SEQUENTIALLY (16 matmuls over all key tiles
    # each) so every group's flush copy + DMA hides under the next group's
    # matmul stream and only the last group's short flush chain remains at
    # the very end.  Costs extra vsc weight-load runs (hidden under the
    # matmul stream).
    ob_tiles = []
    for _oc in range(2):
        O_b = spsum.tile([P, 1024], F32, tag="S")
        ob_tiles.append(O_b)
    prev_mm = last_oa_mm
    # Group order alternates the two PSUM tiles so each quarter's flush copy
    # (a read of the tile) finishes under a DIFFERENT tile's matmul stream --
    # same-tile write-after-read would stall the next group ~0.9us.
    for qt4, (oc, u) in enumerate([(0, 0), (1, 0), (0, 1), (1, 1)]):
        for kt in range(KT_TILES):
            if qt4 == 0 and kt == KT_TILES - 1:
                # The first group's tiles 0..14 fill the ~1us window while the
                # vsc15 chain (accum -> reduce -> recip -> scale) resolves;
                # the half-A close slots in here, then the group's own kt=15.
                pv, pe_t, pkt = pending_g2a
                for coc in range(2):
                    for cu in range(2):
                        cmm = nc.tensor.matmul(
                            oa_tiles[coc][:, cu * 512 : (cu + 1) * 512],
                            lhsT=pv[:],
                            rhs=pe_t[:, coc * 1024 + cu * 512 : coc * 1024 + (cu + 1) * 512],
                            start=False,
                            stop=True,
                        )
                        add_dep_helper(
                            cmm.ins, prev_mm.ins, sync=False,
                            reason="half-A close inside first tail group",
                        )
                        prev_mm = cmm
            mm = nc.tensor.matmul(
                ob_tiles[oc][:, u * 512 : (u + 1) * 512],
                lhsT=v_tiles[kt][:],
                rhs=e_tiles[kt][:, 2048 + oc * 1024 + u * 512 : 2048 + oc * 1024 + (u + 1) * 512],
                start=(kt == 0),
                stop=(kt == KT_TILES - 1),
            )
            add_dep_helper(
                mm.ins, prev_mm.ins, sync=False,
                reason="sequential bank groups in tail",
            )
            prev_mm = mm
        # This quarter's flush + DMA hide under the next quarter's matmuls.
        lo = 2048 + oc * 1024 + u * 512
        src = ob_tiles[oc][:, u * 512 : (u + 1) * 512]
        dst = oacc[:, lo : lo + 512]
        if qt4 % 2 == 0:
            nc.scalar.copy(out=dst, in_=src)
        else:
            nc.vector.tensor_copy(out=dst, in_=src)
        nc.sync.dma_start(out_d[:, lo : lo + 512], dst)
        if qt4 == 0:
            flush_oc(oa_tiles[0], 0)
            flush_oc(oa_tiles[1], 1024)


def build_bass(repeat=1, loop=False):
    nc = bass.Bass("TRN2", target_bir_lowering=False, debug=False)
    qt_d = nc.dram_tensor("qt", [P, N], F16, kind="ExternalInput").ap()
    kt_d = nc.dram_tensor("kt", [P, NK], F16, kind="ExternalInput").ap()
    v_d = nc.dram_tensor("v", [NK, D], F16, kind="ExternalInput").ap()
    out_d = nc.dram_tensor("out_t", [P, N], F16, kind="ExternalOutput").ap()

    with tile.TileContext(nc) as tc:
        with (
            tc.tile_pool(name="big", bufs=1) as big,
            tc.tile_pool(name="epool", bufs=1) as epool,
            tc.tile_pool(name="small", bufs=2) as small,
            tc.tile_pool(name="spsum", bufs=2, space="PSUM") as spsum,
            tc.tile_pool(name="opsum", bufs=2, space="PSUM") as opsum,
        ):
            def body():
                emit_body(
                    nc,
                    tc,
                    (big, epool, small, spsum, opsum),
                    (qt_d, kt_d, v_d, out_d),
                )

            if loop and repeat > 1:
                with tc.For_i(
                    0, repeat, 1,
                    hint_engines=(mybir.EngineType.PE, mybir.EngineType.Activation),
                ):
                    body()
            else:
                for _ in range(repeat):
                    body()
    return nc


def legalize_waits(nc, max_waits=1):
    """Hoist excess semaphore waits into standalone EventSemaphore ops.

    The walrus codegen for several engine instruction structs accepts only a
    single sync-wait command; Tile sometimes emits more.  Executing the extra
    waits in a preceding same-engine EventSemaphore is semantically identical
    (the engine runs its stream in order).
    """
    for fn in nc.m.functions:
        for blk in fn.blocks:
            out = []
            for inst in blk.instructions:
                si = inst.sync_info
                if (
                    si is not None
                    and si.on_wait
                    and len(si.on_wait) > max_waits
                    and inst.opcode != "EventSemaphore"
                ):
                    waits = list(si.on_wait)
                    extra, keep = waits[:-max_waits], waits[-max_waits:]
                    for n, w in enumerate(extra):
                        out.append(
                            mybir.InstEventSemaphore(
                                name=f"{inst.name}_prewait{n}",
                                engine=inst.engine,
                                ins=[],
                                outs=[],
                                sync_info=mybir.SyncInfo(on_wait=[w], on_update=[]),
                            )
                        )
                    si.on_wait = keep
                out.append(inst)
            blk.instructions = out
    return nc


_NC_CACHE = {}


def _get_nc(repeat=1, **kw):
    key = ("nc", repeat, tuple(sorted(kw.items())))
    if key not in _NC_CACHE:
        _NC_CACHE[key] = legalize_waits(build_bass(repeat, **kw))
    return _NC_CACHE[key]


def kernel(q, k, v):
    q = np.asarray(q, dtype=np.float32)
    k = np.asarray(k, dtype=np.float32)
    v = np.asarray(v, dtype=np.float32)

    in_maps = []
    for c in range(8):
        b, h = c // 2, c % 2
        # order='C' fuses the transpose-copy and the fp16 cast in one pass.
        in_maps.append(
            {
                "qt": q[b].T.astype(np.float16, order="C"),
                "kt": k[b, h * NK : (h + 1) * NK].T.astype(np.float16, order="C"),
                "v": v[b, h * NK : (h + 1) * NK].astype(np.float16, order="C"),
            }
        )

    nc = _get_nc()
    res = run_bass_kernel_spmd(nc, in_maps, list(range(8))).results

    out = np.empty((B, N, D), dtype=np.float32)
    for b in range(B):
        out[b] = (
            res[2 * b]["out_t"].astype(np.float32)
            + res[2 * b + 1]["out_t"].astype(np.float32)
        ).T
    return out
